# revision 1
# baseline (speedup 1.0000x reference)
"""Trainium2 Bass kernel for nn_SSLModel (dual-branch 3-layer GCN + segment-max pool + MLP head).

Strategy (8 NeuronCores):
  - Cores 0-3 run branch s, cores 4-7 run branch t (same SPMD program, different data).
  - Within a branch group of 4 cores, nodes/edges are sharded by destination block.
  - Nodes are repacked so every graph occupies L (=128*m) padded rows; 128-row blocks
    never straddle graphs, which makes pooling a segmented reduce.
  - gcn_conv(x) = Ahat @ (x @ W) + b is computed aggregation-first:
      Y = Ahat @ x  (gather rows of x by edge src via indirect DMA, then multiply by
      host-precomputed block one-hot "T matrices" on the tensor engine = scatter-add)
      Z = Y @ W + b (+ relu)
  - All PE traffic is bf16 (fp32 PSUM accumulate): T matrices, gathered features,
    weights and stored activations. Rel err vs fp32 reference ~5e-3.
  - Per-block chunk counts are exact (max over the 8 cores per block slot, so the
    SPMD program shape is shared), not padded to a global max.
  - AllGather (4-core groups) of activations between convs; conv3 output is produced
    transposed so pooling is a free-axis segmented max; one 8-core AllGather combines
    pools; every core computes the small MLP head redundantly.
"""
import numpy as np
from ml_dtypes import bfloat16

from contextlib import ExitStack

try:
    import concourse.bass  # noqa: F401
except ImportError:
    import sys
    sys.path.insert(0, "/opt/trn_rl_repo")

import concourse.bass as bass
import concourse.tile as tile
from concourse import bacc, mybir
from concourse.masks import make_identity

N_NODES = 10000
E_EDGES = 160000
G_GRAPHS = 128
D_IN, D1, D2, D3, DH, D_OUT = 128, 512, 1024, 2048, 1024, 1317
D_OUT_PAD = 1320
NCORES = 8
CPBRANCH = 4  # cores per branch
GROUPS4 = [[0, 1, 2, 3], [4, 5, 6, 7]]
GROUPS8 = [[0, 1, 2, 3, 4, 5, 6, 7]]

import os as _os_mod
AG2_SPLIT = int(_os_mod.environ.get("KAGSPLIT", "1"))
SB_BUFS = int(_os_mod.environ.get("KBUFS", "2"))

f32 = mybir.dt.float32
bf16 = mybir.dt.bfloat16
i32 = mybir.dt.int32


# ----------------------------------------------------------------------------- host prep
def _remap_split(pid, PC):
    """Map padded row id -> its row in the split-AllGather act layout.

    AG piece h concatenates the 4 cores' h-th slices at offset h*4*P, so
    pid (core-major) moves to piece-major: h*4*P + core*P + (r % P).
    """
    if AG2_SPLIT == 1:
        return pid
    P = PC // AG2_SPLIT
    core = pid // PC
    r = pid % PC
    return (r // P) * 4 * P + core * P + (r % P)


def _preprocess(edge_index, batch, L):
    """Pack one branch's edges into dst-block-sorted chunk layout."""
    src = np.asarray(edge_index[0], dtype=np.int64)
    dst = np.asarray(edge_index[1], dtype=np.int64)
    batch = np.asarray(batch, dtype=np.int64)
    n = batch.shape[0]
    counts = np.bincount(batch, minlength=G_GRAPHS)
    order = np.argsort(batch, kind="stable")
    rank = np.empty(n, np.int64)
    starts = np.concatenate([[0], np.cumsum(counts)[:-1]])
    rank[order] = np.arange(n) - starts[batch[order]]
    pid = batch * L + rank  # padded row id of each node

    src_all = np.concatenate([src, np.arange(n)])  # self loops appended
    dst_all = np.concatenate([dst, np.arange(n)])
    deg = np.bincount(dst_all, minlength=n).astype(np.float64)  # >= 1 (self loop)
    dinv = 1.0 / np.sqrt(deg)
    norm = (dinv[src_all] * dinv[dst_all]).astype(np.float32)

    # remap of padded src ids for the split act2 AllGather layout:
    # AG piece h concatenates [c0 piece h, c1 piece h, ...] at offset h*4*P
    pdst = pid[dst_all]
    blk = pdst // 128
    row = pdst % 128
    eorder = np.argsort(blk, kind="stable")
    nblocks = G_GRAPHS * (L // 128)
    bc = np.bincount(blk[eorder], minlength=nblocks)
    return dict(
        blk=blk[eorder], row=row[eorder], norm=norm[eorder],
        src=src_all[eorder], psrc=pid[src_all[eorder]],
        counts=counts, bc=bc, L=L,
    )


def _pack_ragged(pp, cbs, offs, TOT, NB, slot_of):
    """Per-core ragged chunk packing for one branch.

    slot_of[k][i] = program slot of core k's local block i (sorted by size so
    the 8-core per-slot max stays tight). Returns T [4,128,TOT*128] bf16,
    idx1/idx2 [4,128,TOT] i32, flags [4,1,NB*128] bf16 (slot order).
    """
    L = pp["L"]
    m = L // 128
    blk = pp["blk"]
    bstart = np.concatenate([[0], np.cumsum(pp["bc"])[:-1]])
    j = np.arange(len(blk)) - bstart[blk]
    c = j // 128
    e = j % 128
    k = blk // NB
    i = blk % NB
    s = slot_of[k, i]
    col_t = (offs[s] + c) * 128 + pp["row"]
    col_i = offs[s] + c
    T = np.zeros((CPBRANCH, 128, TOT * 128), np.float32)
    I1 = np.zeros((CPBRANCH, 128, TOT), np.int32)
    I2 = np.zeros((CPBRANCH, 128, TOT), np.int32)
    T[k, e, col_t] = pp["norm"]
    I1[k, e, col_i] = pp["src"]
    I2[k, e, col_i] = pp["psrc"]
    flags = np.zeros((G_GRAPHS * m, 128), np.float32)
    for g in range(G_GRAPHS):
        cnt = int(pp["counts"][g])
        for mb in range(m):
            fill = min(max(cnt - mb * 128, 0), 128)
            flags[g * m + mb, :fill] = 1.0
    flags = flags.reshape(CPBRANCH, NB, 128)
    fl_slot = np.zeros_like(flags)
    for kk in range(CPBRANCH):
        fl_slot[kk, slot_of[kk]] = flags[kk]
    return (T.astype(bfloat16), I1, I2,
            fl_slot.reshape(CPBRANCH, 1, NB * 128).astype(bfloat16))


# ----------------------------------------------------------------------------- program
def build_nc(cbs, m, repeat=1, stages=5, has_bias=True):
    """Build the SPMD Bass program. cbs = per-block-slot chunk counts (len NB)."""
    cbs = list(cbs)
    NB = len(cbs)
    assert NB == (G_GRAPHS // CPBRANCH) * m
    offs = np.concatenate([[0], np.cumsum(cbs)[:-1]]).astype(int)
    TOT = int(np.sum(cbs))
    NPAD = CPBRANCH * NB * 128  # padded rows per branch

    nc = bacc.Bacc("TRN2", target_bir_lowering=False, debug=False, num_devices=NCORES)

    x_in = nc.dram_tensor("x", [N_NODES, D_IN], bf16, kind="ExternalInput")
    T_in = nc.dram_tensor("Tm", [128, TOT * 128], bf16, kind="ExternalInput")
    idx1_in = nc.dram_tensor("idx1", [128, TOT], i32, kind="ExternalInput")
    idx2_in = nc.dram_tensor("idx2", [128, TOT], i32, kind="ExternalInput")
    flags_in = nc.dram_tensor("flags", [1, NB * 128], bf16, kind="ExternalInput")
    W1_in = nc.dram_tensor("W1", [D_IN, D1], bf16, kind="ExternalInput")
    b1_in = nc.dram_tensor("b1", [1, D1], bf16, kind="ExternalInput")
    W2_in = nc.dram_tensor("W2", [D1, D2], bf16, kind="ExternalInput")
    b2_in = nc.dram_tensor("b2", [1, D2], bf16, kind="ExternalInput")
    W3_in = nc.dram_tensor("W3", [D2, D3], bf16, kind="ExternalInput")
    b3_in = nc.dram_tensor("b3", [1, D3], bf16, kind="ExternalInput")
    Wl1_in = nc.dram_tensor("Wl1", [D3, DH], bf16, kind="ExternalInput")
    bl1_in = nc.dram_tensor("bl1", [1, DH], bf16, kind="ExternalInput")
    Wl2_in = nc.dram_tensor("Wl2", [DH, D_OUT_PAD], bf16, kind="ExternalInput")
    bl2_in = nc.dram_tensor("bl2", [1, D_OUT_PAD], bf16, kind="ExternalInput")
    prs_in = nc.dram_tensor("prs", [128, 1], i32, kind="ExternalInput")
    prt_in = nc.dram_tensor("prt", [128, 1], i32, kind="ExternalInput")
    W12_in = nc.dram_tensor("W12", [D_IN, D2], bf16, kind="ExternalInput")

    out_z = nc.dram_tensor("out_z", [G_GRAPHS, D_OUT], f32, kind="ExternalOutput")
    out_sig = nc.dram_tensor("out_sig", [G_GRAPHS, D_OUT], f32, kind="ExternalOutput")

    import os as _os
    _kdebug = bool(_os.environ.get("KDEBUG"))
    act1_loc = nc.dram_tensor("act1_loc", [NB * 128, D1], bf16)
    act1 = nc.dram_tensor("act1", [NPAD, D1], bf16)
    u_loc = nc.dram_tensor("u_loc", [NB * 128, D_IN], bf16)
    u_all = nc.dram_tensor("u_all", [NPAD, D_IN], bf16)
    vT_loc = nc.dram_tensor("vT_loc", [NB * 128, 128], bf16)
    vT_all = nc.dram_tensor("vT_all", [NPAD, 128], bf16)
    act2_loc = nc.dram_tensor("act2_loc", [NB * 128, D2], bf16)
    act2 = nc.dram_tensor("act2", [NPAD, D2], bf16)
    pool_loc = nc.dram_tensor("pool_loc", [G_GRAPHS // CPBRANCH, D3], bf16)
    dbg = {}
    if _kdebug:
        dbg["act1_loc"] = nc.dram_tensor("dbg_act1", [NB * 128, D1], bf16, kind="ExternalOutput")
        dbg["act2_loc"] = nc.dram_tensor("dbg_act2", [NB * 128, D2], bf16, kind="ExternalOutput")
        dbg["pool_loc"] = nc.dram_tensor("dbg_pool", [G_GRAPHS // CPBRANCH, D3], bf16, kind="ExternalOutput")
    pool_all = nc.dram_tensor("pool_all", [2 * G_GRAPHS, D3], bf16)

    GPG = G_GRAPHS // CPBRANCH  # graphs per core (32)

    with tile.TileContext(nc) as tc:
        with ExitStack() as rctx:
            res = rctx.enter_context(tc.tile_pool(name="res", bufs=1))
            ident = res.tile([128, 128], bf16)
            make_identity(nc, ident[:])
            ones_row = res.tile([1, 128], bf16)
            nc.vector.memset(ones_row[:], 1.0)

            dram = dict(
                x_in=x_in, T_in=T_in, idx1_in=idx1_in, idx2_in=idx2_in,
                flags_in=flags_in, W1_in=W1_in, b1_in=b1_in, W2_in=W2_in,
                b2_in=b2_in, W3_in=W3_in, b3_in=b3_in, Wl1_in=Wl1_in,
                bl1_in=bl1_in, Wl2_in=Wl2_in, bl2_in=bl2_in,
                prs_in=prs_in, prt_in=prt_in,
                act1_loc=act1_loc, act1=act1, act2_loc=act2_loc, act2=act2,
                u_loc=u_loc, u_all=u_all, W12_in=W12_in,
                vT_loc=vT_loc, vT_all=vT_all,
                pool_loc=pool_loc, pool_all=pool_all,
                out_z=out_z, out_sig=out_sig, dbg=dbg,
            )
            for _rep in range(repeat):
                _emit_pipeline(nc, tc, cbs, offs, NB, m, dram, ident, ones_row,
                               GPG, stages=stages, has_bias=has_bias)

    nc.compile()
    return nc


def _dummy_outputs(nc, tc, dram):
    with ExitStack() as ctx:
        sb = ctx.enter_context(tc.tile_pool(name="dout", bufs=1))
        t = sb.tile([128, D_OUT], f32)
        nc.vector.memset(t[:], 0.0)
        nc.sync.dma_start(out=dram["out_z"][:, :], in_=t[:])
        nc.sync.dma_start(out=dram["out_sig"][:, :], in_=t[:])


def _emit_pipeline(nc, tc, cbs, offs, NB, m, dram, ident, ones_row, GPG,
                   stages=5, has_bias=True):
    AG = "AllGather"
    BYP = mybir.AluOpType.bypass
    x_in, T_in = dram["x_in"], dram["T_in"]
    idx1_in, idx2_in = dram["idx1_in"], dram["idx2_in"]
    act1_loc, act1 = dram["act1_loc"], dram["act1"]
    act2_loc, act2 = dram["act2_loc"], dram["act2"]
    pool_loc, pool_all = dram["pool_loc"], dram["pool_all"]
    CBM = max(cbs)

    if not has_bias:
        _emit_fused12(nc, tc, cbs, offs, NB, dram, ident, CBM, stages)
        if stages < 3:
            _dummy_outputs(nc, tc, dram)
            return
        _emit_conv3_head(nc, tc, cbs, offs, NB, m, dram, ident, ones_row, GPG,
                         stages, has_bias, CBM, skip_ag2=True)
        return

    # ---------------- conv1: x[10000,128] -> act1_loc[NB*128, 512] (no relu)
    with ExitStack() as ctx:
        wb = ctx.enter_context(tc.tile_pool(name="c1wb", bufs=1))
        sb = ctx.enter_context(tc.tile_pool(name="c1sb", bufs=SB_BUFS))
        ps = ctx.enter_context(tc.tile_pool(name="c1ps", bufs=2, space="PSUM"))
        W1_sb = wb.tile([128, D1], bf16)
        nc.sync.dma_start(out=W1_sb[:], in_=dram["W1_in"][:, :])
        b1_sb = wb.tile([1, D1], bf16)
        nc.sync.dma_start(out=b1_sb[:], in_=dram["b1_in"][:, :])
        for b in range(NB):
            cb, off = cbs[b], offs[b]
            T_sb = sb.tile([128, CBM * 128], bf16, tag="T", name="T_sb")
            nc.sync.dma_start(out=T_sb[:, :cb * 128],
                              in_=T_in[:, off * 128:(off + cb) * 128])
            ix = sb.tile([128, CBM], i32, tag="ix", name="ix")
            nc.sync.dma_start(out=ix[:, :cb], in_=idx1_in[:, off:off + cb])
            G = sb.tile([128, CBM * D_IN], bf16, tag="G", name="G")
            for c in range(cb):
                nc.gpsimd.indirect_dma_start(
                    out=G[:, c * D_IN:(c + 1) * D_IN], out_offset=None, in_=x_in[:, :],
                    in_offset=bass.IndirectOffsetOnAxis(ap=ix[:, c:c + 1], axis=0))
            y_ps = ps.tile([128, D_IN], f32, tag="y", name="y_ps")
            for c in range(cb):
                nc.tensor.matmul(
                    out=y_ps[:], lhsT=T_sb[:, c * 128:(c + 1) * 128],
                    rhs=G[:, c * D_IN:(c + 1) * D_IN],
                    start=(c == 0), stop=(c == cb - 1))
            y_sb = sb.tile([128, D_IN], bf16, tag="ys", name="y_sb")
            nc.scalar.copy(out=y_sb[:], in_=y_ps[:])
            t_ps = ps.tile([128, 128], bf16, tag="tp", name="t_ps")
            nc.tensor.transpose(out=t_ps[:], in_=y_sb[:], identity=ident[:])
            yt = sb.tile([128, D_IN], bf16, tag="yt", name="yt")
            nc.scalar.copy(out=yt[:], in_=t_ps[:])
            z_ps = ps.tile([128, D1], f32, tag="z", name="z_ps")
            nc.tensor.matmul(out=z_ps[:], lhsT=yt[:],
                             rhs=W1_sb[:], start=True, stop=not has_bias)
            if has_bias:
                nc.tensor.matmul(out=z_ps[:], lhsT=ones_row[:], rhs=b1_sb[:],
                                 start=False, stop=True)
            z_sb = sb.tile([128, D1], bf16, tag="zs", name="z_sb")
            nc.scalar.copy(out=z_sb[:], in_=z_ps[:])
            nc.sync.dma_start(out=act1_loc[b * 128:(b + 1) * 128, :], in_=z_sb[:])

    if dram["dbg"]:
        nc.sync.dma_start(out=dram["dbg"]["act1_loc"][:, :], in_=act1_loc[:, :])
    if stages < 2:
        _dummy_outputs(nc, tc, dram)
        return
    P1 = (NB * 128) // AG2_SPLIT
    for h in range(AG2_SPLIT):
        nc.gpsimd.collective_compute(
            AG, BYP, ins=[act1_loc[h * P1:(h + 1) * P1, :]],
            outs=[act1[h * 4 * P1:(h + 1) * 4 * P1, :]],
            replica_groups=GROUPS4)

    # ---------------- conv2: act1 -> act2_loc[NB*128, 1024] (relu)
    with ExitStack() as ctx:
        wb = ctx.enter_context(tc.tile_pool(name="c2wb", bufs=1))
        sb = ctx.enter_context(tc.tile_pool(name="c2sb", bufs=SB_BUFS))
        ps = ctx.enter_context(tc.tile_pool(name="c2ps", bufs=2, space="PSUM"))
        W2_sb = [wb.tile([128, D2], bf16, tag=f"W2_{k}", name=f"W2_{k}") for k in range(D1 // 128)]
        for k in range(D1 // 128):
            nc.sync.dma_start(out=W2_sb[k][:], in_=dram["W2_in"][k * 128:(k + 1) * 128, :])
        b2_sb = wb.tile([1, D2], bf16)
        nc.sync.dma_start(out=b2_sb[:], in_=dram["b2_in"][:, :])
        for b in range(NB):
            cb, off = cbs[b], offs[b]
            T_sb = sb.tile([128, CBM * 128], bf16, tag="T", name="T_sb")
            nc.sync.dma_start(out=T_sb[:, :cb * 128],
                              in_=T_in[:, off * 128:(off + cb) * 128])
            ix = sb.tile([128, CBM], i32, tag="ix", name="ix")
            nc.sync.dma_start(out=ix[:, :cb], in_=idx2_in[:, off:off + cb])
            G = sb.tile([128, CBM * D1], bf16, tag="G", name="G")
            for c in range(cb):
                nc.gpsimd.indirect_dma_start(
                    out=G[:, c * D1:(c + 1) * D1], out_offset=None, in_=act1[:, :],
                    in_offset=bass.IndirectOffsetOnAxis(ap=ix[:, c:c + 1], axis=0))
            y_ps = ps.tile([128, D1], f32, tag="y", name="y_ps")
            for c in range(cb):
                nc.tensor.matmul(
                    out=y_ps[:], lhsT=T_sb[:, c * 128:(c + 1) * 128],
                    rhs=G[:, c * D1:(c + 1) * D1],
                    start=(c == 0), stop=(c == cb - 1))
            y_sb = sb.tile([128, D1], bf16, tag="ys", name="y_sb")
            nc.scalar.copy(out=y_sb[:], in_=y_ps[:])
            yt = sb.tile([128, D1], bf16, tag="yt", name="yt")
            tpb = ps.tile([128, D1], bf16, tag="tp", name="tpb")
            for k in range(D1 // 128):
                nc.tensor.transpose(out=tpb[:, k * 128:(k + 1) * 128],
                                    in_=y_sb[:, k * 128:(k + 1) * 128],
                                    identity=ident[:])
            nc.scalar.copy(out=yt[:], in_=tpb[:])
            z_ps = ps.tile([128, D2], f32, tag="z", name="z_ps")
            KD1 = D1 // 128
            for nn in range(D2 // 512):
                nsl = slice(nn * 512, (nn + 1) * 512)
                for k in range(KD1):
                    nc.tensor.matmul(
                        out=z_ps[:, nsl], lhsT=yt[:, k * 128:(k + 1) * 128],
                        rhs=W2_sb[k][:, nsl],
                        start=(k == 0), stop=(not has_bias and k == KD1 - 1))
                if has_bias:
                    nc.tensor.matmul(out=z_ps[:, nsl], lhsT=ones_row[:],
                                     rhs=b2_sb[:, nsl], start=False, stop=True)
            z_sb = sb.tile([128, D2], bf16, tag="zs", name="z_sb")
            nc.scalar.activation(out=z_sb[:], in_=z_ps[:],
                                 func=mybir.ActivationFunctionType.Relu)
            nc.sync.dma_start(out=act2_loc[b * 128:(b + 1) * 128, :], in_=z_sb[:])

    if dram["dbg"]:
        nc.sync.dma_start(out=dram["dbg"]["act2_loc"][:, :], in_=act2_loc[:, :])
    if stages < 3:
        _dummy_outputs(nc, tc, dram)
        return
    _emit_conv3_head(nc, tc, cbs, offs, NB, m, dram, ident, ones_row, GPG,
                     stages, has_bias, CBM)


def _emit_agg_pass(nc, tc, name, cbs, offs, NB, CBM, src_tensor, idx_tensor,
                   T_in, dst_loc, dram, ident, W_sb=None, relu=False, DW=None):
    """One aggregation pass: dst_loc[slot] = Ahat_block @ src (+ @W if W_sb)."""
    D_src = src_tensor.shape[1]
    with ExitStack() as ctx:
        sb = ctx.enter_context(tc.tile_pool(name=f"{name}sb", bufs=SB_BUFS))
        ps = ctx.enter_context(tc.tile_pool(name=f"{name}ps", bufs=2, space="PSUM"))
        # 4 slots share one PSUM tile (disjoint 128-col slices) so the
        # PSUM->SBUF copy and the store batch 4-wide
        GRP = 16
        CBM4 = max(sum(cbs[q * GRP:(q + 1) * GRP]) for q in range(NB // GRP))
        for q in range(NB // GRP):
            g0 = offs[q * GRP]
            gw = offs[q * GRP + GRP - 1] + cbs[q * GRP + GRP - 1] - g0
            T_sb = sb.tile([128, CBM4 * 128], bf16, tag="T", name="T_sb")
            nc.sync.dma_start(out=T_sb[:, :gw * 128],
                              in_=T_in[:, g0 * 128:(g0 + gw) * 128])
            ix = sb.tile([128, CBM4], i32, tag="ix", name="ix")
            nc.sync.dma_start(out=ix[:, :gw], in_=idx_tensor[:, g0:g0 + gw])
            y_ps = ps.tile([128, GRP * D_src], f32, tag="y", name="y_ps")
            for j in range(GRP):
                b = q * GRP + j
                cb, ro = cbs[b], offs[b] - g0
                G = sb.tile([128, CBM * D_src], bf16, tag="G", name="G")
                for c in range(cb):
                    nc.gpsimd.indirect_dma_start(
                        out=G[:, c * D_src:(c + 1) * D_src], out_offset=None,
                        in_=src_tensor[:, :],
                        in_offset=bass.IndirectOffsetOnAxis(
                            ap=ix[:, ro + c:ro + c + 1], axis=0))
                ysl = y_ps[:, j * D_src:(j + 1) * D_src]
                for c in range(cb):
                    if W_sb is None:
                        nc.tensor.matmul(
                            out=ysl, lhsT=T_sb[:, (ro + c) * 128:(ro + c + 1) * 128],
                            rhs=G[:, c * D_src:(c + 1) * D_src],
                            start=(c == 0), stop=(c == cb - 1))
                    else:
                        # aggregate directly transposed: Y^T = sum G_c^T @ T_c
                        nc.tensor.matmul(
                            out=ysl, lhsT=G[:, c * D_src:(c + 1) * D_src],
                            rhs=T_sb[:, (ro + c) * 128:(ro + c + 1) * 128],
                            start=(c == 0), stop=(c == cb - 1))
            y_sb = sb.tile([128, GRP * D_src], bf16, tag="ys", name="y_sb")
            nc.scalar.copy(out=y_sb[:], in_=y_ps[:])
            nc.sync.dma_start(
                out=dst_loc[q * GRP * 128:(q + 1) * GRP * 128, :]
                    .rearrange("(j p) c -> p j c", j=GRP),
                in_=y_sb[:].rearrange("p (j c) -> p j c", j=GRP))


def _emit_fused12(nc, tc, cbs, offs, NB, dram, ident, CBM, stages):
    """conv1+conv2 fused via linearity (zero biases):
    act2 = relu(Ahat (Ahat x) W12), W12 = W1 @ W2 host-side."""
    AG = "AllGather"
    BYP = mybir.AluOpType.bypass
    _emit_agg_pass(nc, tc, "fA", cbs, offs, NB, CBM, dram["x_in"],
                   dram["idx1_in"], dram["T_in"], dram["u_loc"], dram, ident)
    if stages < 2:
        return
    nc.gpsimd.collective_compute(AG, BYP, ins=[dram["u_loc"][:, :]],
                                 outs=[dram["u_all"][:, :]],
                                 replica_groups=GROUPS4)
    _emit_agg_pass(nc, tc, "fB", cbs, offs, NB, CBM, dram["u_all"],
                   dram["idx2_in"], dram["T_in"], dram["vT_loc"], dram,
                   ident, W_sb=True)
    nc.gpsimd.collective_compute(AG, BYP, ins=[dram["vT_loc"][:, :]],
                                 outs=[dram["vT_all"][:, :]],
                                 replica_groups=GROUPS4)
    # replicated transform: act2[slot] = relu(vT[slot]^T @ W12) for ALL slots
    with ExitStack() as wctx:
        wb = wctx.enter_context(tc.tile_pool(name="fWb", bufs=1))
        sb = wctx.enter_context(tc.tile_pool(name="fTs", bufs=SB_BUFS))
        ps = wctx.enter_context(tc.tile_pool(name="fTp", bufs=2, space="PSUM"))
        W12_sb = wb.tile([128, D2], bf16)
        nc.sync.dma_start(out=W12_sb[:], in_=dram["W12_in"][:, :])
        for s16 in range(CPBRANCH * NB // 16):
            vt = sb.tile([128, 16 * 128], bf16, tag="vt", name="vt")
            nc.sync.dma_start(
                out=vt[:].rearrange("p (g c) -> p g c", g=16),
                in_=dram["vT_all"][s16 * 2048:(s16 + 1) * 2048, :]
                    .rearrange("(g p) c -> p g c", g=16))
            for g in range(16):
                s = s16 * 16 + g
                z_ps = ps.tile([128, D2], f32, tag="z", name="z_ps")
                for nn in range(D2 // 512):
                    nc.tensor.matmul(out=z_ps[:, nn * 512:(nn + 1) * 512],
                                     lhsT=vt[:, g * 128:(g + 1) * 128],
                                     rhs=W12_sb[:, nn * 512:(nn + 1) * 512],
                                     start=True, stop=True)
                z_sb = sb.tile([128, D2], bf16, tag="zs", name="z_sb")
                nc.scalar.activation(out=z_sb[:], in_=z_ps[:],
                                     func=mybir.ActivationFunctionType.Relu)
                nc.sync.dma_start(out=dram["act2"][s * 128:(s + 1) * 128, :], in_=z_sb[:])


def _emit_conv3_head(nc, tc, cbs, offs, NB, m, dram, ident, ones_row, GPG,
                     stages, has_bias, CBM, skip_ag2=False):
    AG = "AllGather"
    BYP = mybir.AluOpType.bypass
    T_in = dram["T_in"]
    idx2_in = dram["idx2_in"]
    act2_loc, act2 = dram["act2_loc"], dram["act2"]
    pool_loc, pool_all = dram["pool_loc"], dram["pool_all"]
    if skip_ag2 or _os_mod.environ.get("KNOAG2"):
        pass  # fused path: act2 already materialized locally on every core
    else:
        P2 = (NB * 128) // AG2_SPLIT
        for h in range(AG2_SPLIT):
            nc.gpsimd.collective_compute(
                AG, BYP, ins=[act2_loc[h * P2:(h + 1) * P2, :]],
                outs=[act2[h * 4 * P2:(h + 1) * 4 * P2, :]],
                replica_groups=GROUPS4)

    # ---------------- conv3 + pooling (transposed transform, no relu before max)
    with ExitStack() as ctx:
        wb = ctx.enter_context(tc.tile_pool(name="c3wb", bufs=1))
        sb = ctx.enter_context(tc.tile_pool(name="c3sb", bufs=SB_BUFS))
        ps = ctx.enter_context(tc.tile_pool(name="c3ps", bufs=2, space="PSUM"))
        pool_res = ctx.enter_context(tc.tile_pool(name="poolres", bufs=1))
        W3_all = wb.tile([128, (D2 // 128) * D3], bf16, name="W3_all")
        nc.sync.dma_start(
            out=W3_all[:].rearrange("p (k c) -> p k c", c=D3),
            in_=dram["W3_in"][:, :].rearrange("(k p) c -> p k c", p=128))
        W3_sb = [W3_all[:, k * D3:(k + 1) * D3] for k in range(D2 // 128)]
        b3_sb = wb.tile([1, D3], bf16)
        nc.sync.dma_start(out=b3_sb[:], in_=dram["b3_in"][:, :])
        poolT = [pool_res.tile([128, NB], bf16, tag=f"poolT{oc}", name=f"poolT{oc}") for oc in range(D3 // 128)]
        KD2 = D2 // 128
        CBM4 = max(sum(cbs[q * 4:(q + 1) * 4]) for q in range(NB // 4))
        for q in range(NB // 4):
            g0 = offs[q * 4]
            gw = offs[q * 4 + 3] + cbs[q * 4 + 3] - g0
            T_sb = sb.tile([128, CBM4 * 128], bf16, tag="T", name="T_sb")
            nc.sync.dma_start(out=T_sb[:, :gw * 128],
                              in_=T_in[:, g0 * 128:(g0 + gw) * 128])
            ix = sb.tile([128, CBM4], i32, tag="ix", name="ix")
            nc.sync.dma_start(out=ix[:, :gw], in_=idx2_in[:, g0:g0 + gw])
            ytg = sb.tile([128, KD2 * 512], bf16, tag="ytg", name="ytg")
            for j in range(4):
                b = q * 4 + j
                cb, ro = cbs[b], offs[b] - g0
                G = sb.tile([128, CBM * D2], bf16, tag="G", name="G")
                for c in range(cb):
                    nc.gpsimd.indirect_dma_start(
                        out=G[:, c * D2:(c + 1) * D2], out_offset=None, in_=act2[:, :],
                        in_offset=bass.IndirectOffsetOnAxis(
                            ap=ix[:, ro + c:ro + c + 1], axis=0))
                y_ps = ps.tile([128, D2], f32, tag="y", name="y_ps")
                for c in range(cb):
                    for nn in range(D2 // 512):
                        nc.tensor.matmul(
                            out=y_ps[:, nn * 512:(nn + 1) * 512],
                            lhsT=T_sb[:, (ro + c) * 128:(ro + c + 1) * 128],
                            rhs=G[:, c * D2 + nn * 512: c * D2 + (nn + 1) * 512],
                            start=(c == 0), stop=(c == cb - 1))
                y_sb = sb.tile([128, D2], bf16, tag="ys", name="y_sb")
                nc.scalar.copy(out=y_sb[:], in_=y_ps[:])
                # ytg layout: [128 d2-rows, k-chunk*512 + j*128 + node]
                nc.sync.dma_start_transpose(
                    out=ytg[:].rearrange("p (k j n) -> p k j n", k=KD2, j=4)[:, :, j, :],
                    in_=y_sb[:])
            if has_bias:
                fl = sb.tile([1, 512], bf16, tag="fl", name="fl")
                nc.sync.dma_start(out=fl[:], in_=dram["flags_in"][0:1, q * 512:(q + 1) * 512])
            for oc in range(D3 // 128):
                zt_ps = ps.tile([128, 512], f32, tag="zt", name="zt_ps")
                for k in range(KD2):
                    nc.tensor.matmul(
                        out=zt_ps[:], lhsT=W3_sb[k][:, oc * 128:(oc + 1) * 128],
                        rhs=ytg[:, k * 512:(k + 1) * 512],
                        start=(k == 0), stop=(not has_bias and k == KD2 - 1))
                if has_bias:
                    nc.tensor.matmul(out=zt_ps[:], lhsT=b3_sb[:, oc * 128:(oc + 1) * 128],
                                     rhs=fl[:], start=False, stop=True)
                nc.vector.tensor_reduce(
                    out=poolT[oc][:, q * 4:(q + 1) * 4],
                    in_=zt_ps[:].rearrange("p (g n) -> p g n", n=128),
                    axis=mybir.AxisListType.X, op=mybir.AluOpType.max)

        # fold m blocks per graph (no-op when m == 1), transpose pool to [32, 2048]
        pool_sb = pool_res.tile([32, D3], bf16, tag="pool_sb", name="pool_sb")
        for oc in range(D3 // 128):
            if m > 1:
                pg = pool_res.tile([128, GPG], bf16, tag="pg", name="pg")
                nc.vector.tensor_reduce(
                    out=pg[:], in_=poolT[oc][:].rearrange("p (g mm) -> p g mm", mm=m),
                    axis=mybir.AxisListType.X, op=mybir.AluOpType.max)
                src_t = pg
            else:
                src_t = poolT[oc]
            # transpose [128, 32] -> [32, 128]
            ptile = ps.tile([32, 128], bf16, tag="tp", name="ptile")
            nc.tensor.transpose(out=ptile[:], in_=src_t[:], identity=ident[:])
            nc.scalar.copy(out=pool_sb[:, oc * 128:(oc + 1) * 128], in_=ptile[:])
        nc.sync.dma_start(out=pool_loc[:, :], in_=pool_sb[:])

    if dram["dbg"]:
        nc.sync.dma_start(out=dram["dbg"]["pool_loc"][:, :], in_=pool_loc[:, :])
    if stages < 4:
        _dummy_outputs(nc, tc, dram)
        return
    nc.gpsimd.collective_compute(AG, BYP, ins=[pool_loc[:, :]], outs=[pool_all[:, :]],
                                 replica_groups=GROUPS8)
    if stages < 5:
        _dummy_outputs(nc, tc, dram)
        return

    # ---------------- head (every core computes it; graded output from core 0)
    with ExitStack() as ctx:
        sb = ctx.enter_context(tc.tile_pool(name="hsb", bufs=2))
        wsb = ctx.enter_context(tc.tile_pool(name="hwsb", bufs=1))
        ps = ctx.enter_context(tc.tile_pool(name="hps", bufs=2, space="PSUM"))
        bl1_sb = wsb.tile([1, DH], bf16)
        nc.sync.dma_start(out=bl1_sb[:], in_=dram["bl1_in"][:, :])
        bl2_sb = wsb.tile([1, D_OUT_PAD], bf16)
        nc.sync.dma_start(out=bl2_sb[:], in_=dram["bl2_in"][:, :])
        wl1_all = wsb.tile([128, (D3 // 128) * DH], bf16, name="wl1_all")
        nc.sync.dma_start(
            out=wl1_all[:].rearrange("p (k c) -> p k c", c=DH),
            in_=dram["Wl1_in"][:, :].rearrange("(k p) c -> p k c", p=128))
        wl1 = [wl1_all[:, k * DH:(k + 1) * DH] for k in range(D3 // 128)]
        wl2_all = wsb.tile([128, (DH // 128) * D_OUT_PAD], bf16, name="wl2_all")
        nc.sync.dma_start(
            out=wl2_all[:].rearrange("p (k c) -> p k c", c=D_OUT_PAD),
            in_=dram["Wl2_in"][:, :].rearrange("(k p) c -> p k c", p=128))
        wl2 = [wl2_all[:, k * D_OUT_PAD:(k + 1) * D_OUT_PAD] for k in range(DH // 128)]
        pr_s = wsb.tile([128, 1], i32, name="pr_s")
        nc.sync.dma_start(out=pr_s[:], in_=dram["prs_in"][:, :])
        pr_t = wsb.tile([128, 1], i32, name="pr_t")
        nc.sync.dma_start(out=pr_t[:], in_=dram["prt_in"][:, :])
        za = sb.tile([128, D3], bf16, tag="za", name="za")
        nc.gpsimd.indirect_dma_start(
            out=za[:], out_offset=None, in_=pool_all[:, :],
            in_offset=bass.IndirectOffsetOnAxis(ap=pr_s[:, 0:1], axis=0))
        zb = sb.tile([128, D3], bf16, tag="zb", name="zb")
        nc.gpsimd.indirect_dma_start(
            out=zb[:], out_offset=None, in_=pool_all[:, :],
            in_offset=bass.IndirectOffsetOnAxis(ap=pr_t[:, 0:1], axis=0))
        z_sb = sb.tile([128, D3], bf16, tag="zsum", name="z_sb")
        nc.vector.tensor_add(out=z_sb[:], in0=za[:], in1=zb[:])
        zT = sb.tile([128, D3], bf16, tag="zT", name="zT")
        nc.sync.dma_start_transpose(
            out=zT[:].rearrange("p (k n) -> p k n", n=128), in_=z_sb[:])
        KD3 = D3 // 128
        h_ps = ps.tile([128, DH], f32, tag="h", name="h_ps")
        for nn in range(DH // 512):
            nsl = slice(nn * 512, (nn + 1) * 512)
            for k in range(KD3):
                nc.tensor.matmul(out=h_ps[:, nsl], lhsT=zT[:, k * 128:(k + 1) * 128],
                                 rhs=wl1[k][:, nsl], start=(k == 0),
                                 stop=(not has_bias and k == KD3 - 1))
            if has_bias:
                nc.tensor.matmul(out=h_ps[:, nsl], lhsT=ones_row[:], rhs=bl1_sb[:, nsl],
                                 start=False, stop=True)
        h_sb = sb.tile([128, DH], bf16, tag="hs", name="h_sb")
        nc.scalar.activation(out=h_sb[:], in_=h_ps[:],
                             func=mybir.ActivationFunctionType.Relu)
        hT = sb.tile([128, DH], bf16, tag="hT", name="hT")
        nc.sync.dma_start_transpose(
            out=hT[:].rearrange("p (k n) -> p k n", n=128), in_=h_sb[:])
        KDH = DH // 128
        z_out = sb.tile([128, D_OUT], f32, tag="zo", name="z_out")
        sig = sb.tile([128, D_OUT], f32, tag="sg", name="sig")
        sls = [(0, 512, 512), (512, 1024, 1024), (1024, D_OUT, D_OUT_PAD)]
        for (lo, hi, hp) in sls:
            o_ps = ps.tile([128, 512], f32, tag="o", name="o_ps")
            for k in range(KDH):
                nc.tensor.matmul(out=o_ps[:, :hp - lo], lhsT=hT[:, k * 128:(k + 1) * 128],
                                 rhs=wl2[k][:, lo:hp], start=(k == 0),
                                 stop=(not has_bias and k == KDH - 1))
            if has_bias:
                nc.tensor.matmul(out=o_ps[:, :hp - lo], lhsT=ones_row[:],
                                 rhs=bl2_sb[:, lo:hp], start=False, stop=True)
            nc.scalar.copy(out=z_out[:, lo:hi], in_=o_ps[:, :hi - lo])
        nc.scalar.activation(out=sig[:], in_=z_out[:],
                             func=mybir.ActivationFunctionType.Sigmoid)
        nc.sync.dma_start(out=dram["out_z"][:, :], in_=z_out[:])
        nc.sync.dma_start(out=dram["out_sig"][:, :], in_=sig[:])


# ----------------------------------------------------------------------------- driver
_PROGRAM_CACHE = {}


def _get_program(cbs, m, repeat=1, stages=5, has_bias=True):
    import os
    stages = int(os.environ.get("KSTAGES", stages))
    key = (tuple(cbs), m, repeat, stages, AG2_SPLIT, SB_BUFS,
           bool(os.environ.get("KNOAG2")), has_bias)
    if key not in _PROGRAM_CACHE:
        _PROGRAM_CACHE[key] = build_nc(cbs, m, repeat=repeat, stages=stages,
                                       has_bias=has_bias)
    return _PROGRAM_CACHE[key]


def make_in_maps(x_s, x_t, W_enc1, b_enc1, W_enc2, b_enc2,
                 W_r1g1, b_r1g1, W_r1g2, b_r1g2,
                 W_r2g1, b_r2g1, W_r2g2, b_r2g2,
                 W_l1, b_l1, W_l2, b_l2,
                 edge_index_s, edge_index_t, xs_batch, xt_batch):
    """Host preprocessing -> (in_maps, cbs, m)."""
    cs = np.bincount(np.asarray(xs_batch, np.int64), minlength=G_GRAPHS)
    ct = np.bincount(np.asarray(xt_batch, np.int64), minlength=G_GRAPHS)
    L = 128 * int(np.ceil(max(cs.max(), ct.max(), 1) / 128))
    m = L // 128
    pp_s = _preprocess(np.asarray(edge_index_s), xs_batch, L)
    pp_t = _preprocess(np.asarray(edge_index_t), xt_batch, L)

    NBLK = G_GRAPHS * m
    NB = NBLK // CPBRANCH

    # per-slot chunk counts: max over the 8 cores so the SPMD program is
    # shared. When m == 1, each core's blocks are sorted by descending size
    # first so the per-slot max stays near the per-core sum (the pooling
    # permutation is undone by an indirect gather in the head).
    bc8 = np.vstack([pp_s["bc"].reshape(CPBRANCH, NB),
                     pp_t["bc"].reshape(CPBRANCH, NB)])
    cb8 = np.maximum(-(-bc8 // 128), 1).astype(int)
    if m == 1 and not _os_mod.environ.get("KNOSORT"):
        order8 = np.argsort(-cb8, axis=1, kind="stable")   # slot -> local block
        slot8 = np.argsort(order8, axis=1)                 # local block -> slot
    else:
        order8 = np.tile(np.arange(NB), (2 * CPBRANCH, 1))
        slot8 = order8.copy()
    cbs = np.take_along_axis(cb8, order8, axis=1).max(axis=0)
    offs = np.concatenate([[0], np.cumsum(cbs)[:-1]]).astype(int)
    TOT = int(cbs.sum())

    # remap padded row ids (conv2/conv3 gather indices) to the slot-permuted
    # act layout: act rows are stored per core in slot order
    def remap_pid(pid, slot_of):
        PC = NB * 128
        core = pid // PC
        r = pid % PC
        i = r // 128
        return core * PC + slot_of[core, r // 128] * 128 + (r % 128)

    PC_ = NB * 128
    pp_s["psrc"] = _remap_split(remap_pid(pp_s["psrc"], slot8[:CPBRANCH]), PC_)
    pp_t["psrc"] = _remap_split(remap_pid(pp_t["psrc"], slot8[CPBRANCH:]), PC_)

    T_s, idx1_s, idx2_s, flags_s = _pack_ragged(pp_s, cbs, offs, TOT, NB,
                                                slot8[:CPBRANCH])
    T_t, idx1_t, idx2_t, flags_t = _pack_ragged(pp_t, cbs, offs, TOT, NB,
                                                slot8[CPBRANCH:])

    # head unpermute indices: graph g of branch s lives at pool_all row
    # core(g)*NB + slot(g); branch t offset by G_GRAPHS
    g = np.arange(G_GRAPHS)
    prs = (g // NB) * NB + slot8[:CPBRANCH][g // NB, g % NB]
    prt = G_GRAPHS + (g // NB) * NB + slot8[CPBRANCH:][g // NB, g % NB]
    prs = prs.astype(np.int32).reshape(128, 1)
    prt = prt.astype(np.int32).reshape(128, 1)

    has_bias = any(
        float(np.abs(np.asarray(b, np.float32)).max()) > 0
        for b in (b_enc1, b_enc2, b_r1g1, b_r1g2, b_r2g1, b_r2g2, b_l1, b_l2))

    def b16(a):
        return np.ascontiguousarray(np.asarray(a, np.float32).astype(bfloat16))

    def padw(W, b):
        Wp = np.zeros((DH, D_OUT_PAD), np.float32)
        Wp[:, :D_OUT] = np.asarray(W, np.float32)
        bp = np.zeros((1, D_OUT_PAD), np.float32)
        bp[0, :D_OUT] = np.asarray(b, np.float32).ravel()
        return b16(Wp), b16(bp)

    Wl2p, bl2p = padw(W_l2, b_l2)
    common = dict(
        Wl1=b16(W_l1), bl1=b16(np.asarray(b_l1).reshape(1, -1)),
        Wl2=Wl2p, bl2=bl2p,
    )
    W12_s = b16(np.asarray(W_enc1, np.float32) @ np.asarray(W_r1g1, np.float32))
    W12_t = b16(np.asarray(W_enc2, np.float32) @ np.asarray(W_r2g1, np.float32))
    branch = {
        "s": dict(x=b16(x_s), W12=W12_s,
                  W1=b16(W_enc1), b1=b16(np.asarray(b_enc1).reshape(1, -1)),
                  W2=b16(W_r1g1), b2=b16(np.asarray(b_r1g1).reshape(1, -1)),
                  W3=b16(W_r1g2), b3=b16(np.asarray(b_r1g2).reshape(1, -1)),
                  T=T_s, idx1=idx1_s, idx2=idx2_s, flags=flags_s),
        "t": dict(x=b16(x_t), W12=W12_t,
                  W1=b16(W_enc2), b1=b16(np.asarray(b_enc2).reshape(1, -1)),
                  W2=b16(W_r2g1), b2=b16(np.asarray(b_r2g1).reshape(1, -1)),
                  W3=b16(W_r2g2), b3=b16(np.asarray(b_r2g2).reshape(1, -1)),
                  T=T_t, idx1=idx1_t, idx2=idx2_t, flags=flags_t),
    }
    in_maps = []
    for core in range(NCORES):
        br = branch["s" if core < CPBRANCH else "t"]
        k = core % CPBRANCH
        in_maps.append(dict(
            x=br["x"], prs=prs, prt=prt,
            Tm=np.ascontiguousarray(br["T"][k]),
            idx1=np.ascontiguousarray(br["idx1"][k]),
            idx2=np.ascontiguousarray(br["idx2"][k]),
            flags=np.ascontiguousarray(br["flags"][k]),
            W1=br["W1"], b1=br["b1"], W2=br["W2"], b2=br["b2"], W12=br["W12"],
            W3=br["W3"], b3=br["b3"],
            **common,
        ))
    return in_maps, tuple(int(c) for c in cbs), m, has_bias


def kernel(**inputs):
    from concourse import bass2jax
    in_maps, cbs, m, has_bias = make_in_maps(**inputs)
    nc = _get_program(cbs, m, has_bias=has_bias)
    results = bass2jax.run_bass_via_pjrt(nc, in_maps, n_cores=NCORES)
    z = results[0]["out_z"]
    sig = results[0]["out_sig"]
    return (z, sig)

